# revision 19
# baseline (speedup 1.0000x reference)
"""Trainium2 Bass kernel for a 12-qubit batched PennyLane-style circuit.

Circuit (per batch sample), 4 layers:
  - data-encoding RY,RX,RZ,RY per wire (per-sample angles) followed by a
    fixed Rot per wire  -> folded on host into ONE 2x2 SU(2) gate G[l,q,b]
  - CRot entangling ring CRot(q, q+1 mod 12), fixed per layer.
Then <Z_i> for each of the 12 wires.

Distribution: pure data parallel over the batch. 4096 samples -> 8 cores
x 512 samples; each core holds its 512x4096 complex statevector in SBUF as
fp32 re/im planes, batch on partitions (4 tiles of 128 samples).

Gate application is elementwise: for a 1q gate on qubit q the statevector
pairs (s0, s1) sit at free-dim stride 2^(11-q); y = [[a,b],[c,d]] @ [s0,s1]
is computed with tensor_scalar / scalar_tensor_tensor chains whose scalars
are per-partition (= per-sample) coefficient columns, DMA'd in once.
Layer 1's 12 per-sample gates acting on |0..0> are replaced by a direct
Kronecker build of the product state (much cheaper).
"""

import numpy as np

import concourse.bass as bass
import concourse.bacc as bacc
import concourse.mybir as mybir
from concourse.tile import TileContext
from concourse.bass_utils import run_bass_kernel_spmd

F32 = mybir.dt.float32
ALU = mybir.AluOpType

N_QUBITS = 12
N_LAYERS = 4
DIM = 4096            # 2**12
B_FULL = 4096
N_CORES = 8
B_CORE = B_FULL // N_CORES   # 512
NBT = B_CORE // 128          # 4 batch tiles of 128 samples

# coefficient plane order per gate (12 per-partition scalars)
#  a=[0,0] b=[0,1] c=[1,0] d=[1,1] of the 2x2 complex gate
CO_ARE, CO_AIM, CO_MAIM, CO_BRE, CO_BIM, CO_MBIM, \
    CO_CRE, CO_CIM, CO_MCIM, CO_DRE, CO_DIM, CO_MDIM = range(12)
NCO = 12

GCO_W = N_LAYERS * N_QUBITS * NCO * NBT    # per-sample gate coeffs
CCO_W = N_LAYERS * N_QUBITS * NCO          # crot coeffs (same for all samples)

# ---------------------------------------------------------------------------
# engine plan knobs: weighted round-robin per unit kind.
# 'pe' = TensorE diag-matmul path, 'dve'/'gps' = elementwise chain engines.
PLAN_PS = ("pe", "pe", "pe", "dve", "pe", "pe", "dve", "pe", "dve", "pe",
           "pe", "dve")
PLAN_CROT = ("pe", "dve", "pe", "pe", "dve", "pe", "pe", "dve", "pe",
             "pe", "pe", "dve")
PROD_ON_ACT = True     # chain-start products of dve natives -> ScalarE
COPY_ON = "act"        # copybacks of chain natives: "same" | "act"
SQUARES_ON_ACT = True  # observable squares on ScalarE
EVICT_ROT = ("act", "dve", "act")  # psum evictions rotate over these
F32R = mybir.dt.float32r

# ---------------------------------------------------------------------------
# host-side gate algebra (numpy, trivially cheap vs the device work)
# ---------------------------------------------------------------------------


def _rz(t):
    e = np.exp(-0.5j * t)
    z = np.zeros_like(e)
    return np.stack([np.stack([e, z], -1), np.stack([z, np.conj(e)], -1)], -2)


def _ry(t):
    c = np.cos(t / 2).astype(np.complex128)
    s = np.sin(t / 2).astype(np.complex128)
    return np.stack([np.stack([c, -s], -1), np.stack([s, c], -1)], -2)


def _rx(t):
    c = np.cos(t / 2).astype(np.complex128)
    s = np.sin(t / 2).astype(np.complex128)
    return np.stack([np.stack([c, -1j * s], -1), np.stack([-1j * s, c], -1)], -2)


def _rot(phi, theta, omega):
    # PennyLane Rot = RZ(omega) @ RY(theta) @ RZ(phi)
    return _rz(omega) @ _ry(theta) @ _rz(phi)


def _coef_planes(g):
    """g: [..., 2, 2] complex -> [..., 12] float32 coefficient planes."""
    a, b = g[..., 0, 0], g[..., 0, 1]
    c, d = g[..., 1, 0], g[..., 1, 1]
    cols = [a.real, a.imag, -a.imag, b.real, b.imag, -b.imag,
            c.real, c.imag, -c.imag, d.real, d.imag, -d.imag]
    return np.stack(cols, -1).astype(np.float32)


def _host_coeffs(x, q_params_rot, q_params_enta):
    """Returns (gco [L,Q,12,B] f32, cco [L,Q,12] f32)."""
    x = np.asarray(x, np.float64)
    pr = np.asarray(q_params_rot, np.float64)
    pe = np.asarray(q_params_enta, np.float64)

    # per-sample encoding gate per wire: RY(x3) RZ(x2) RX(x1) RY(x0)
    enc = np.einsum('qbij,qbjk->qbik',
                    _ry(x[:, 3, :].T),
                    np.einsum('qbij,qbjk->qbik', _rz(x[:, 2, :].T),
                              np.einsum('qbij,qbjk->qbik',
                                        _rx(x[:, 1, :].T), _ry(x[:, 0, :].T))))
    rot = _rot(pr[..., 0], pr[..., 1], pr[..., 2])      # [L,Q,2,2]
    g = np.einsum('lqij,qbjk->lqbik', rot, enc)         # [L,Q,B,2,2]
    cr = _rot(pe[..., 0], pe[..., 1], pe[..., 2])       # [L,Q,2,2]

    gco = np.moveaxis(_coef_planes(g), -1, 2)           # [L,Q,12,B]
    cco = _coef_planes(cr)                              # [L,Q,12]
    return gco.astype(np.float32), cco.astype(np.float32)


# ---------------------------------------------------------------------------
# bass program
# ---------------------------------------------------------------------------


class _Prog:
    def __init__(self):
        nc = bacc.Bacc("TRN2", target_bir_lowering=False, debug=False)
        self.nc = nc
        self.gco_d = nc.declare_dram_parameter("gcoef", [128, GCO_W], F32,
                                               isOutput=False)
        self.cco_d = nc.declare_dram_parameter("ccoef", [128, CCO_W], F32,
                                               isOutput=False)
        self.idn_d = nc.declare_dram_parameter("ident", [128, 128], F32,
                                               isOutput=False)
        self.z_d = nc.declare_dram_parameter("z", [B_CORE, N_QUBITS], F32,
                                             isOutput=True)
        self._uctr = {"ps": 0, "crot": 0}
        self._ectr = 0
        with TileContext(nc) as tc:
            self.tc = tc
            with tc.tile_pool(name="main", bufs=1) as pool, \
                    tc.tile_pool(name="dpool", bufs=32) as dpool, \
                    tc.tile_pool(name="psum", bufs=4, space="PSUM") as ppool:
                self.dpool = dpool
                self.ppool = ppool
                # state: bt-major, then comp (0=re 1=im), then 4096 amplitudes
                self.ST = pool.tile([128, NBT * 2 * DIM], F32R, tag="state")
                self.GC = pool.tile([128, GCO_W], F32, tag="gc")
                self.CC = pool.tile([128, CCO_W], F32, tag="cc")
                self.I128 = pool.tile([128, 128], F32, tag="ident")
                # per-chain-engine temp sets (avoid cross-engine serialization)
                self.TS = [
                    [pool.tile([128, 1024], F32, name=f"t{s}{i}",
                               tag=f"t{s}{i}") for i in range(4)]
                    for s in range(2)
                ]
                self._tsctr = 0
                self.ZT = [pool.tile([128, 16], F32, name=f"z{bt}",
                                     tag=f"z{bt}") for bt in range(NBT)]

                nc.sync.dma_start(out=self.GC[:], in_=self.gco_d[:])
                nc.sync.dma_start(out=self.CC[:], in_=self.cco_d[:])
                nc.sync.dma_start(out=self.I128[:], in_=self.idn_d[:])

                self._emit_circuit()

                for bt in range(NBT):
                    nc.sync.dma_start(
                        out=self.z_d[bt * 128:(bt + 1) * 128, :],
                        in_=self.ZT[bt][:, 0:N_QUBITS])
        nc.compile()

    # ---- AP helpers -----------------------------------------------------

    def plane(self, bt, comp):
        """[128, 4096] AP of one re/im plane of one batch tile."""
        off = (bt * 2 + comp) * DIM
        return self.ST[:, off:off + DIM]

    def half(self, bt, comp, q, bit):
        """[128, n, s] AP: amplitudes with qubit q's bit == bit."""
        s = 1 << (11 - q)
        p = self.plane(bt, comp).rearrange("p (a c r) -> p a c r", c=2, r=s)
        return p[:, :, bit, :]

    def crot_half(self, bt, comp, c, t, bit):
        """AP over amplitudes with ctrl bit c == 1 and target bit t == bit."""
        if c < t:      # adjacent, c = t-1
            st = 1 << (11 - t)
            p = self.plane(bt, comp).rearrange(
                "p (a cc tt r) -> p a cc tt r", cc=2, tt=2, r=st)
            return p[:, :, 1, bit, :]
        else:          # wrap: c=11 (LSB), t=0 (MSB)
            p = self.plane(bt, comp).rearrange(
                "p (tt a cc) -> p tt a cc", tt=2, cc=2)
            return p[:, bit, :, 1]

    def gco(self, bt, l, q, ci):
        idx = (((l * N_QUBITS + q) * NCO) + ci) * NBT + bt
        return self.GC[:, idx:idx + 1]

    def cco(self, l, q, ci):
        idx = (l * N_QUBITS + q) * NCO + ci
        return self.CC[:, idx:idx + 1]

    @staticmethod
    def _chunk(view, idx, csz):
        """csz-wide column chunk of a slice-AP shaped [128, w] or [128,n,s]."""
        shp = view.shape[1:]
        if len(shp) == 1:
            return view[:, idx * csz:(idx + 1) * csz]
        n, s = shp
        if s >= csz:
            m = s // csz
            return view[:, idx // m, (idx % m) * csz:(idx % m + 1) * csz]
        na = csz // s
        return view[:, idx * na:(idx + 1) * na, :]

    @staticmethod
    def _tview(tile, view, csz):
        """View of a [128,1024] temp matching the chunk geometry of view."""
        shp = view.shape[1:]
        if len(shp) == 1 or shp[1] >= csz:
            return tile[:, 0:csz]
        s = shp[1]
        return tile[:, 0:csz].rearrange("p (a r) -> p a r", r=s)

    # ---- gate emission --------------------------------------------------

    def _chains(self, eng, s0re, s0im, s1re, s1im, co, temps, cidx=None):
        """The 4 mult-add chains of a 2x2 complex gate on given slices.
        Returns temp APs (y0re, y0im, y1re, y1im)."""
        nc = self.nc
        t0, t1, t2, t3 = temps
        AF = mybir.ActivationFunctionType

        def start(t, src, ci):
            if PROD_ON_ACT:
                nc.scalar.activation(t, src, AF.Copy, scale=co(ci))
            else:
                eng.tensor_scalar(t, src, co(ci), None, ALU.mult)

        start(t0, s0re, CO_ARE)
        eng.scalar_tensor_tensor(t0, s0im, co(CO_MAIM), t0, ALU.mult, ALU.add)
        eng.scalar_tensor_tensor(t0, s1re, co(CO_BRE), t0, ALU.mult, ALU.add)
        eng.scalar_tensor_tensor(t0, s1im, co(CO_MBIM), t0, ALU.mult, ALU.add)
        start(t1, s0im, CO_ARE)
        eng.scalar_tensor_tensor(t1, s0re, co(CO_AIM), t1, ALU.mult, ALU.add)
        eng.scalar_tensor_tensor(t1, s1im, co(CO_BRE), t1, ALU.mult, ALU.add)
        eng.scalar_tensor_tensor(t1, s1re, co(CO_BIM), t1, ALU.mult, ALU.add)
        start(t2, s0re, CO_CRE)
        eng.scalar_tensor_tensor(t2, s0im, co(CO_MCIM), t2, ALU.mult, ALU.add)
        eng.scalar_tensor_tensor(t2, s1re, co(CO_DRE), t2, ALU.mult, ALU.add)
        eng.scalar_tensor_tensor(t2, s1im, co(CO_MDIM), t2, ALU.mult, ALU.add)
        start(t3, s0im, CO_CRE)
        eng.scalar_tensor_tensor(t3, s0re, co(CO_CIM), t3, ALU.mult, ALU.add)
        eng.scalar_tensor_tensor(t3, s1im, co(CO_DRE), t3, ALU.mult, ALU.add)
        eng.scalar_tensor_tensor(t3, s1re, co(CO_DIM), t3, ALU.mult, ALU.add)
        return t0, t1, t2, t3

    def _gate_native(self, ename, slices, co, width):
        """Chain-engine gate: emitted in 1024-wide column passes."""
        nc = self.nc
        eng = nc.vector
        tset = self.TS[self._tsctr % 2]
        self._tsctr += 1
        s0re, s0im, s1re, s1im = slices
        csz = min(width, 1024)
        for h in range(width // csz):
            subs = [self._chunk(v, h, csz) for v in slices]
            temps = [self._tview(t, subs[0], csz) for t in tset]
            y = self._chains(eng, *subs, co, temps)
            for dst, yy in zip(subs, y):
                if COPY_ON == "act":
                    nc.scalar.copy(dst, yy)
                else:
                    eng.tensor_copy(out=dst, in_=yy)

    def _build_diags(self, co):
        """12 diag weight tiles for a gate, builds split DVE/ACT."""
        nc = self.nc
        AF = mybir.ActivationFunctionType
        D = {}
        for ci in range(NCO):
            d = self.dpool.tile([128, 128], F32R, name="dg", tag="dg")
            if (self._ectr + ci) % 2 == 0:
                nc.scalar.activation(d[:], self.I128[:], AF.Copy,
                                     scale=co(ci))
            else:
                nc.vector.tensor_scalar(d[:], self.I128[:], co(ci),
                                        None, ALU.mult)
            D[ci] = d
        return D

    def _gate_mm(self, slices, co, width, D=None):
        """TensorE diag-matmul gate with PSUM accumulation."""
        nc = self.nc
        s0re, s0im, s1re, s1im = slices
        CSZ = 512
        nch = width // CSZ
        if D is None:
            D = self._build_diags(co)
        halves = [
            (s0re, s0im,
             [(CO_ARE, ((0, s0re), (1, s0im))),
              (CO_MAIM, ((0, s0im),)), (CO_AIM, ((1, s0re),)),
              (CO_BRE, ((0, s1re), (1, s1im))),
              (CO_MBIM, ((0, s1im),)), (CO_BIM, ((1, s1re),))]),
            (s1re, s1im,
             [(CO_CRE, ((0, s0re), (1, s0im))),
              (CO_MCIM, ((0, s0im),)), (CO_CIM, ((1, s0re),)),
              (CO_DRE, ((0, s1re), (1, s1im))),
              (CO_MDIM, ((0, s1im),)), (CO_DIM, ((1, s1re),))]),
        ]
        for p0 in range(0, nch, 2):
            chs = [c for c in (p0, p0 + 1) if c < nch]
            pw = len(chs) * CSZ
            psums = {}
            nterm = {}
            for hi, (ore, oim, groups) in enumerate(halves):
                for (ci, uses) in groups:
                    for (comp, rhs_view) in uses:
                        for c in chs:
                            key = (hi, comp)
                            if key not in psums:
                                psums[key] = self.ppool.tile(
                                    [128, pw], F32, name="ps", tag="ps")
                            k = nterm.get((key, c), 0)
                            nc.tensor.matmul(
                                out=psums[key][:, (c - p0) * CSZ:
                                              (c - p0 + 1) * CSZ],
                                lhsT=D[ci][:],
                                rhs=self._chunk(rhs_view, c, CSZ),
                                start=(k == 0), stop=(k == 3))
                            nterm[(key, c)] = k + 1
            for hi, (ore, oim, groups) in enumerate(halves):
                for comp, dst in ((0, ore), (1, oim)):
                    dstap = self._chunk(dst, p0 // 2, pw)
                    src = psums[(hi, comp)][:]
                    if len(dstap.shape) > 2:
                        src = src.rearrange("p (a r) -> p a r",
                                            r=dstap.shape[-1])
                    ev = EVICT_ROT[self._ectr % len(EVICT_ROT)]
                    self._ectr += 1
                    if ev == "act":
                        nc.scalar.copy(dstap, src)
                    else:
                        nc.vector.tensor_copy(out=dstap, in_=src)

    def _gate_1q(self, bt, l, q):
        plan = PLAN_PS[self._uctr["ps"] % len(PLAN_PS)]
        self._uctr["ps"] += 1
        slices = (self.half(bt, 0, q, 0), self.half(bt, 1, q, 0),
                  self.half(bt, 0, q, 1), self.half(bt, 1, q, 1))
        co = lambda ci: self.gco(bt, l, q, ci)
        if plan == "pe":
            self._gate_mm(slices, co, DIM // 2)
        else:
            self._gate_native(plan, slices, co, DIM // 2)

    def _crot_site(self, l, c):
        plan = PLAN_CROT[self._uctr["crot"] % len(PLAN_CROT)]
        self._uctr["crot"] += 1
        t = (c + 1) % N_QUBITS
        co = lambda ci: self.cco(l, c, ci)
        D = self._build_diags(co) if plan == "pe" else None
        for bt in range(NBT):
            slices = (self.crot_half(bt, 0, c, t, 0),
                      self.crot_half(bt, 1, c, t, 0),
                      self.crot_half(bt, 0, c, t, 1),
                      self.crot_half(bt, 1, c, t, 1))
            if plan == "pe":
                self._gate_mm(slices, co, DIM // 4, D=D)
            else:
                self._gate_native(plan, slices, co, DIM // 4)

    def _kron_init(self, bt):
        """Build layer-1 post-1q-phase product state directly:
        state = kron_q (G[0,q] @ e0), i.e. per qubit the column (a, c)."""
        nc = self.nc
        eng = nc.vector
        re = self.plane(bt, 0)
        im = self.plane(bt, 1)
        co = lambda q, ci: self.gco(bt, 0, q, ci)
        t0 = self.TS[0][0]
        t1 = self.TS[0][1]

        eng.tensor_copy(out=re[:, 0:1], in_=co(11, CO_ARE))
        eng.tensor_copy(out=im[:, 0:1], in_=co(11, CO_AIM))
        eng.tensor_copy(out=re[:, 1:2], in_=co(11, CO_CRE))
        eng.tensor_copy(out=im[:, 1:2], in_=co(11, CO_CIM))
        w = 2
        for q in range(10, -1, -1):
            csz = min(w, 1024)
            for k in range(w // csz):
                sl = slice(k * csz, (k + 1) * csz)
                su = slice(w + k * csz, w + (k + 1) * csz)
                ore, oim = re[:, sl], im[:, sl]
                tt0, tt1 = t0[:, 0:csz], t1[:, 0:csz]
                # upper half <- (c) * old  (written before old is clobbered)
                eng.tensor_scalar(tt0, ore, co(q, CO_CRE), None, ALU.mult)
                eng.scalar_tensor_tensor(re[:, su], oim, co(q, CO_MCIM),
                                         tt0, ALU.mult, ALU.add)
                eng.tensor_scalar(tt1, ore, co(q, CO_CIM), None, ALU.mult)
                eng.scalar_tensor_tensor(im[:, su], oim, co(q, CO_CRE),
                                         tt1, ALU.mult, ALU.add)
                # lower half <- (a) * old, in place
                eng.tensor_scalar(tt0, ore, co(q, CO_ARE), None, ALU.mult)
                eng.tensor_scalar(tt1, ore, co(q, CO_AIM), None, ALU.mult)
                eng.scalar_tensor_tensor(ore, oim, co(q, CO_MAIM),
                                         tt0, ALU.mult, ALU.add)
                eng.scalar_tensor_tensor(oim, oim, co(q, CO_ARE),
                                         tt1, ALU.mult, ALU.add)
            w *= 2

    def _observables(self, bt):
        """probs = re^2+im^2 (overwrites re plane), then the 12 <Z_q>."""
        nc = self.nc
        eng = nc.vector
        AF = mybir.ActivationFunctionType
        re = self.plane(bt, 0)
        im = self.plane(bt, 1)
        t0 = self.TS[0][0]
        t1 = self.TS[0][1]
        for h in range(4):
            sl = slice(h * 1024, (h + 1) * 1024)
            if SQUARES_ON_ACT:
                nc.scalar.activation(t0[:], re[:, sl], AF.Square)
                nc.scalar.activation(t1[:], im[:, sl], AF.Square)
            else:
                eng.tensor_tensor(t0[:], re[:, sl], re[:, sl], ALU.mult)
                eng.tensor_tensor(t1[:], im[:, sl], im[:, sl], ALU.mult)
            eng.tensor_tensor(re[:, sl], t0[:], t1[:], ALU.add)
        # fold out qubits MSB-first; z_q = sum(lo half) - sum(hi half)
        w = DIM
        for q in range(N_QUBITS):
            h = w // 2
            lo, hi = re[:, 0:h], re[:, h:w]
            if h > 1024:  # only q=0: do the diff/reduce in two chunks
                for k in range(2):
                    sk = slice(k * 1024, (k + 1) * 1024)
                    eng.tensor_tensor(t0[:], lo[:, sk], hi[:, sk],
                                      ALU.subtract)
                    eng.tensor_reduce(out=self.ZT[bt][:, 12 + k:13 + k],
                                      in_=t0[:], op=ALU.add,
                                      axis=mybir.AxisListType.X)
                eng.tensor_tensor(self.ZT[bt][:, q:q + 1],
                                  self.ZT[bt][:, 12:13],
                                  self.ZT[bt][:, 13:14], ALU.add)
            else:
                eng.tensor_tensor(t0[:, 0:h], lo, hi, ALU.subtract)
                eng.tensor_reduce(out=self.ZT[bt][:, q:q + 1],
                                  in_=t0[:, 0:h], op=ALU.add,
                                  axis=mybir.AxisListType.X)
            if q < N_QUBITS - 1:
                for k in range(max(1, h // 1024)):
                    sk = slice(k * 1024, min((k + 1) * 1024, h))
                    eng.tensor_tensor(lo[:, sk], lo[:, sk], hi[:, sk],
                                      ALU.add)
            w = h

    def _emit_circuit(self):
        for bt in range(NBT):
            self._kron_init(bt)
        for l in range(N_LAYERS):
            if l > 0:
                for q in range(N_QUBITS):
                    for bt in range(NBT):
                        self._gate_1q(bt, l, q)
            for c in range(N_QUBITS):
                self._crot_site(l, c)
        for bt in range(NBT):
            self._observables(bt)


_PROG_CACHE = None


def _get_prog():
    global _PROG_CACHE
    if _PROG_CACHE is None:
        _PROG_CACHE = _Prog()
    return _PROG_CACHE


def _run(inputs, trace=False):
    x = np.asarray(inputs["x"], np.float32)
    gco, cco = _host_coeffs(x, inputs["q_params_rot"], inputs["q_params_enta"])
    # gco: [L,Q,12,B] -> per-core [128, L*Q*12*NBT]
    in_maps = []
    cco_tile = np.broadcast_to(
        cco.reshape(1, CCO_W), (128, CCO_W)).copy()
    for core in range(N_CORES):
        lo = core * B_CORE
        g = gco[:, :, :, lo:lo + B_CORE]                 # [L,Q,12,512]
        g = g.reshape(N_LAYERS, N_QUBITS, NCO, NBT, 128)  # [L,Q,12,bt,p]
        g = np.ascontiguousarray(np.moveaxis(g, -1, 0))   # [p,L,Q,12,bt]
        in_maps.append({
            "gcoef": g.reshape(128, GCO_W),
            "ccoef": cco_tile,
            "ident": np.eye(128, dtype=np.float32),
        })
    prog = _get_prog()
    res = run_bass_kernel_spmd(prog.nc, in_maps, list(range(N_CORES)),
                               trace=trace)
    z = np.concatenate([res.results[c]["z"] for c in range(N_CORES)], axis=0)
    return z.astype(np.float32), res


def kernel(**inputs):
    z, _ = _run(inputs, trace=False)
    return z


# revision 20
# speedup vs baseline: 1.2147x; 1.2147x over previous
"""Trainium2 Bass kernel for a 12-qubit batched PennyLane-style circuit.

Circuit (per batch sample), 4 layers:
  - data-encoding RY,RX,RZ,RY per wire (per-sample angles) followed by a
    fixed Rot per wire  -> folded on host into ONE 2x2 SU(2) gate G[l,q,b]
  - CRot entangling ring CRot(q, q+1 mod 12), fixed per layer.
Then <Z_i> for each of the 12 wires.

Distribution: pure data parallel over the batch. 4096 samples -> 8 cores
x 512 samples; each core holds its 512x4096 complex statevector in SBUF as
fp32 re/im planes, batch on partitions (4 tiles of 128 samples).

Gate application is elementwise: for a 1q gate on qubit q the statevector
pairs (s0, s1) sit at free-dim stride 2^(11-q); y = [[a,b],[c,d]] @ [s0,s1]
is computed with tensor_scalar / scalar_tensor_tensor chains whose scalars
are per-partition (= per-sample) coefficient columns, DMA'd in once.
Layer 1's 12 per-sample gates acting on |0..0> are replaced by a direct
Kronecker build of the product state (much cheaper).
"""

import numpy as np

import concourse.bass as bass
import concourse.bacc as bacc
import concourse.mybir as mybir
from concourse.tile import TileContext
from concourse.bass_utils import run_bass_kernel_spmd

F32 = mybir.dt.float32
ALU = mybir.AluOpType

N_QUBITS = 12
N_LAYERS = 4
DIM = 4096            # 2**12
B_FULL = 4096
N_CORES = 8
B_CORE = B_FULL // N_CORES   # 512
NBT = B_CORE // 128          # 4 batch tiles of 128 samples

# coefficient plane order per gate (12 per-partition scalars)
#  a=[0,0] b=[0,1] c=[1,0] d=[1,1] of the 2x2 complex gate
CO_ARE, CO_AIM, CO_MAIM, CO_BRE, CO_BIM, CO_MBIM, \
    CO_CRE, CO_CIM, CO_MCIM, CO_DRE, CO_DIM, CO_MDIM = range(12)
NCO = 12

GCO_W = N_LAYERS * N_QUBITS * NCO * NBT    # per-sample gate coeffs
CCO_W = N_LAYERS * N_QUBITS * NCO          # crot coeffs (same for all samples)

# ---------------------------------------------------------------------------
# engine plan knobs: weighted round-robin per unit kind.
# 'pe' = TensorE diag-matmul path, 'dve'/'gps' = elementwise chain engines.
PLAN_PS = ("pe", "pe", "pe", "dve", "pe", "pe", "pe", "pe", "dve", "pe",
           "pe", "dve")
PLAN_CROT = ("pe", "dve", "pe", "pe", "dve", "pe", "pe", "dve", "pe",
             "pe", "pe", "dve")
PROD_ON_ACT = True     # chain-start products of dve natives -> ScalarE
COPY_ON = "act"        # copybacks of chain natives: "same" | "act"
SQUARES_ON_ACT = True  # observable squares on ScalarE
EVICT_ROT = ("act", "dve", "act")  # psum evictions rotate over these
F32R = mybir.dt.float32r

# ---------------------------------------------------------------------------
# host-side gate algebra (numpy, trivially cheap vs the device work)
# ---------------------------------------------------------------------------


def _rz(t):
    e = np.exp(-0.5j * t)
    z = np.zeros_like(e)
    return np.stack([np.stack([e, z], -1), np.stack([z, np.conj(e)], -1)], -2)


def _ry(t):
    c = np.cos(t / 2).astype(np.complex128)
    s = np.sin(t / 2).astype(np.complex128)
    return np.stack([np.stack([c, -s], -1), np.stack([s, c], -1)], -2)


def _rx(t):
    c = np.cos(t / 2).astype(np.complex128)
    s = np.sin(t / 2).astype(np.complex128)
    return np.stack([np.stack([c, -1j * s], -1), np.stack([-1j * s, c], -1)], -2)


def _rot(phi, theta, omega):
    # PennyLane Rot = RZ(omega) @ RY(theta) @ RZ(phi)
    return _rz(omega) @ _ry(theta) @ _rz(phi)


def _coef_planes(g):
    """g: [..., 2, 2] complex -> [..., 12] float32 coefficient planes."""
    a, b = g[..., 0, 0], g[..., 0, 1]
    c, d = g[..., 1, 0], g[..., 1, 1]
    cols = [a.real, a.imag, -a.imag, b.real, b.imag, -b.imag,
            c.real, c.imag, -c.imag, d.real, d.imag, -d.imag]
    return np.stack(cols, -1).astype(np.float32)


def _host_coeffs(x, q_params_rot, q_params_enta):
    """Returns (gco [L,Q,12,B] f32, cco [L,Q,12] f32)."""
    x = np.asarray(x, np.float64)
    pr = np.asarray(q_params_rot, np.float64)
    pe = np.asarray(q_params_enta, np.float64)

    # per-sample encoding gate per wire: RY(x3) RZ(x2) RX(x1) RY(x0)
    enc = np.einsum('qbij,qbjk->qbik',
                    _ry(x[:, 3, :].T),
                    np.einsum('qbij,qbjk->qbik', _rz(x[:, 2, :].T),
                              np.einsum('qbij,qbjk->qbik',
                                        _rx(x[:, 1, :].T), _ry(x[:, 0, :].T))))
    rot = _rot(pr[..., 0], pr[..., 1], pr[..., 2])      # [L,Q,2,2]
    g = np.einsum('lqij,qbjk->lqbik', rot, enc)         # [L,Q,B,2,2]
    cr = _rot(pe[..., 0], pe[..., 1], pe[..., 2])       # [L,Q,2,2]

    gco = np.moveaxis(_coef_planes(g), -1, 2)           # [L,Q,12,B]
    cco = _coef_planes(cr)                              # [L,Q,12]
    return gco.astype(np.float32), cco.astype(np.float32)


# ---------------------------------------------------------------------------
# bass program
# ---------------------------------------------------------------------------


class _Prog:
    def __init__(self):
        nc = bacc.Bacc("TRN2", target_bir_lowering=False, debug=False)
        self.nc = nc
        self.gco_d = nc.declare_dram_parameter("gcoef", [128, GCO_W], F32,
                                               isOutput=False)
        self.cco_d = nc.declare_dram_parameter("ccoef", [128, CCO_W], F32,
                                               isOutput=False)
        self.idn_d = nc.declare_dram_parameter("ident", [128, 128], F32,
                                               isOutput=False)
        self.z_d = nc.declare_dram_parameter("z", [B_CORE, N_QUBITS], F32,
                                             isOutput=True)
        self._uctr = {"ps": 0, "crot": 0}
        self._ectr = 0
        with TileContext(nc) as tc:
            self.tc = tc
            with tc.tile_pool(name="main", bufs=1) as pool, \
                    tc.tile_pool(name="dpool", bufs=32) as dpool, \
                    tc.tile_pool(name="psum", bufs=8, space="PSUM") as ppool:
                self.dpool = dpool
                self.ppool = ppool
                # state: bt-major, then comp (0=re 1=im), then 4096 amplitudes
                self.ST = pool.tile([128, NBT * 2 * DIM], F32R, tag="state")
                self.GC = pool.tile([128, GCO_W], F32, tag="gc")
                self.CC = pool.tile([128, CCO_W], F32, tag="cc")
                self.I128 = pool.tile([128, 128], F32, tag="ident")
                # per-chain-engine temp sets (avoid cross-engine serialization)
                self.TS = [
                    [pool.tile([128, 1024], F32, name=f"t{s}{i}",
                               tag=f"t{s}{i}") for i in range(4)]
                    for s in range(2)
                ]
                self._tsctr = 0
                self.ZT = [pool.tile([128, 16], F32, name=f"z{bt}",
                                     tag=f"z{bt}") for bt in range(NBT)]

                nc.sync.dma_start(out=self.GC[:], in_=self.gco_d[:])
                nc.sync.dma_start(out=self.CC[:], in_=self.cco_d[:])
                nc.sync.dma_start(out=self.I128[:], in_=self.idn_d[:])

                self._emit_circuit()

                for bt in range(NBT):
                    nc.sync.dma_start(
                        out=self.z_d[bt * 128:(bt + 1) * 128, :],
                        in_=self.ZT[bt][:, 0:N_QUBITS])
        nc.compile()

    # ---- AP helpers -----------------------------------------------------

    def plane(self, bt, comp):
        """[128, 4096] AP of one re/im plane of one batch tile."""
        off = (bt * 2 + comp) * DIM
        return self.ST[:, off:off + DIM]

    def half(self, bt, comp, q, bit):
        """[128, n, s] AP: amplitudes with qubit q's bit == bit."""
        s = 1 << (11 - q)
        p = self.plane(bt, comp).rearrange("p (a c r) -> p a c r", c=2, r=s)
        return p[:, :, bit, :]

    def crot_half(self, bt, comp, c, t, bit):
        """AP over amplitudes with ctrl bit c == 1 and target bit t == bit."""
        if c < t:      # adjacent, c = t-1
            st = 1 << (11 - t)
            p = self.plane(bt, comp).rearrange(
                "p (a cc tt r) -> p a cc tt r", cc=2, tt=2, r=st)
            return p[:, :, 1, bit, :]
        else:          # wrap: c=11 (LSB), t=0 (MSB)
            p = self.plane(bt, comp).rearrange(
                "p (tt a cc) -> p tt a cc", tt=2, cc=2)
            return p[:, bit, :, 1]

    def gco(self, bt, l, q, ci):
        idx = (((l * N_QUBITS + q) * NCO) + ci) * NBT + bt
        return self.GC[:, idx:idx + 1]

    def cco(self, l, q, ci):
        idx = (l * N_QUBITS + q) * NCO + ci
        return self.CC[:, idx:idx + 1]

    @staticmethod
    def _chunk(view, idx, csz):
        """csz-wide column chunk of a slice-AP shaped [128, w] or [128,n,s]."""
        shp = view.shape[1:]
        if len(shp) == 1:
            return view[:, idx * csz:(idx + 1) * csz]
        n, s = shp
        if s >= csz:
            m = s // csz
            return view[:, idx // m, (idx % m) * csz:(idx % m + 1) * csz]
        na = csz // s
        return view[:, idx * na:(idx + 1) * na, :]

    @staticmethod
    def _tview(tile, view, csz):
        """View of a [128,1024] temp matching the chunk geometry of view."""
        shp = view.shape[1:]
        if len(shp) == 1 or shp[1] >= csz:
            return tile[:, 0:csz]
        s = shp[1]
        return tile[:, 0:csz].rearrange("p (a r) -> p a r", r=s)

    # ---- gate emission --------------------------------------------------

    def _chains(self, eng, s0re, s0im, s1re, s1im, co, temps, cidx=None):
        """The 4 mult-add chains of a 2x2 complex gate on given slices.
        Returns temp APs (y0re, y0im, y1re, y1im)."""
        nc = self.nc
        t0, t1, t2, t3 = temps
        AF = mybir.ActivationFunctionType

        def start(t, src, ci):
            if PROD_ON_ACT:
                nc.scalar.activation(t, src, AF.Copy, scale=co(ci))
            else:
                eng.tensor_scalar(t, src, co(ci), None, ALU.mult)

        start(t0, s0re, CO_ARE)
        eng.scalar_tensor_tensor(t0, s0im, co(CO_MAIM), t0, ALU.mult, ALU.add)
        eng.scalar_tensor_tensor(t0, s1re, co(CO_BRE), t0, ALU.mult, ALU.add)
        eng.scalar_tensor_tensor(t0, s1im, co(CO_MBIM), t0, ALU.mult, ALU.add)
        start(t1, s0im, CO_ARE)
        eng.scalar_tensor_tensor(t1, s0re, co(CO_AIM), t1, ALU.mult, ALU.add)
        eng.scalar_tensor_tensor(t1, s1im, co(CO_BRE), t1, ALU.mult, ALU.add)
        eng.scalar_tensor_tensor(t1, s1re, co(CO_BIM), t1, ALU.mult, ALU.add)
        start(t2, s0re, CO_CRE)
        eng.scalar_tensor_tensor(t2, s0im, co(CO_MCIM), t2, ALU.mult, ALU.add)
        eng.scalar_tensor_tensor(t2, s1re, co(CO_DRE), t2, ALU.mult, ALU.add)
        eng.scalar_tensor_tensor(t2, s1im, co(CO_MDIM), t2, ALU.mult, ALU.add)
        start(t3, s0im, CO_CRE)
        eng.scalar_tensor_tensor(t3, s0re, co(CO_CIM), t3, ALU.mult, ALU.add)
        eng.scalar_tensor_tensor(t3, s1im, co(CO_DRE), t3, ALU.mult, ALU.add)
        eng.scalar_tensor_tensor(t3, s1re, co(CO_DIM), t3, ALU.mult, ALU.add)
        return t0, t1, t2, t3

    def _gate_native(self, ename, slices, co, width):
        """Chain-engine gate: emitted in 1024-wide column passes."""
        nc = self.nc
        eng = nc.vector
        tset = self.TS[self._tsctr % 2]
        self._tsctr += 1
        s0re, s0im, s1re, s1im = slices
        csz = min(width, 1024)
        for h in range(width // csz):
            subs = [self._chunk(v, h, csz) for v in slices]
            temps = [self._tview(t, subs[0], csz) for t in tset]
            y = self._chains(eng, *subs, co, temps)
            for dst, yy in zip(subs, y):
                if COPY_ON == "act":
                    nc.scalar.copy(dst, yy)
                else:
                    eng.tensor_copy(out=dst, in_=yy)

    def _build_diags(self, co):
        """12 diag weight tiles for a gate, builds split DVE/ACT."""
        nc = self.nc
        AF = mybir.ActivationFunctionType
        D = {}
        for ci in range(NCO):
            d = self.dpool.tile([128, 128], F32R, name="dg", tag="dg")
            if (self._ectr + ci) % 2 == 0:
                nc.scalar.activation(d[:], self.I128[:], AF.Copy,
                                     scale=co(ci))
            else:
                nc.vector.tensor_scalar(d[:], self.I128[:], co(ci),
                                        None, ALU.mult)
            D[ci] = d
        return D

    def _gate_mm(self, slices, co, width, D=None):
        """TensorE diag-matmul gate with PSUM accumulation."""
        nc = self.nc
        s0re, s0im, s1re, s1im = slices
        CSZ = 512
        nch = width // CSZ
        if D is None:
            D = self._build_diags(co)
        halves = [
            (s0re, s0im,
             [(CO_ARE, ((0, s0re), (1, s0im))),
              (CO_MAIM, ((0, s0im),)), (CO_AIM, ((1, s0re),)),
              (CO_BRE, ((0, s1re), (1, s1im))),
              (CO_MBIM, ((0, s1im),)), (CO_BIM, ((1, s1re),))]),
            (s1re, s1im,
             [(CO_CRE, ((0, s0re), (1, s0im))),
              (CO_MCIM, ((0, s0im),)), (CO_CIM, ((1, s0re),)),
              (CO_DRE, ((0, s1re), (1, s1im))),
              (CO_MDIM, ((0, s1im),)), (CO_DIM, ((1, s1re),))]),
        ]
        for p0 in range(0, nch, 2):
            chs = [c for c in (p0, p0 + 1) if c < nch]
            psums = {}
            nterm = {}
            for hi, (ore, oim, groups) in enumerate(halves):
                for (ci, uses) in groups:
                    for (comp, rhs_view) in uses:
                        for c in chs:
                            key = (hi, comp, c)
                            if key not in psums:
                                psums[key] = self.ppool.tile(
                                    [128, CSZ], F32, name="ps", tag="ps")
                            k = nterm.get(key, 0)
                            nc.tensor.matmul(
                                out=psums[key][:],
                                lhsT=D[ci][:],
                                rhs=self._chunk(rhs_view, c, CSZ),
                                start=(k == 0), stop=(k == 3))
                            nterm[key] = k + 1
            for hi, (ore, oim, groups) in enumerate(halves):
                for comp, dst in ((0, ore), (1, oim)):
                    for c in chs:
                        dstap = self._chunk(dst, c, CSZ)
                        src = psums[(hi, comp, c)][:]
                        if len(dstap.shape) > 2:
                            src = src.rearrange("p (a r) -> p a r",
                                                r=dstap.shape[-1])
                        ev = EVICT_ROT[self._ectr % len(EVICT_ROT)]
                        self._ectr += 1
                        if ev == "act":
                            nc.scalar.copy(dstap, src)
                        else:
                            nc.vector.tensor_copy(out=dstap, in_=src)

    def _gate_1q(self, bt, l, q):
        plan = PLAN_PS[self._uctr["ps"] % len(PLAN_PS)]
        self._uctr["ps"] += 1
        slices = (self.half(bt, 0, q, 0), self.half(bt, 1, q, 0),
                  self.half(bt, 0, q, 1), self.half(bt, 1, q, 1))
        co = lambda ci: self.gco(bt, l, q, ci)
        if plan == "pe":
            self._gate_mm(slices, co, DIM // 2)
        else:
            self._gate_native(plan, slices, co, DIM // 2)

    def _crot_site(self, l, c):
        plan = PLAN_CROT[self._uctr["crot"] % len(PLAN_CROT)]
        self._uctr["crot"] += 1
        t = (c + 1) % N_QUBITS
        co = lambda ci: self.cco(l, c, ci)
        D = self._build_diags(co) if plan == "pe" else None
        for bt in range(NBT):
            slices = (self.crot_half(bt, 0, c, t, 0),
                      self.crot_half(bt, 1, c, t, 0),
                      self.crot_half(bt, 0, c, t, 1),
                      self.crot_half(bt, 1, c, t, 1))
            if plan == "pe":
                self._gate_mm(slices, co, DIM // 4, D=D)
            else:
                self._gate_native(plan, slices, co, DIM // 4)

    def _kron_init(self, bt):
        """Build layer-1 post-1q-phase product state directly:
        state = kron_q (G[0,q] @ e0), i.e. per qubit the column (a, c)."""
        nc = self.nc
        eng = nc.vector
        re = self.plane(bt, 0)
        im = self.plane(bt, 1)
        co = lambda q, ci: self.gco(bt, 0, q, ci)
        t0 = self.TS[0][0]
        t1 = self.TS[0][1]

        eng.tensor_copy(out=re[:, 0:1], in_=co(11, CO_ARE))
        eng.tensor_copy(out=im[:, 0:1], in_=co(11, CO_AIM))
        eng.tensor_copy(out=re[:, 1:2], in_=co(11, CO_CRE))
        eng.tensor_copy(out=im[:, 1:2], in_=co(11, CO_CIM))
        w = 2
        for q in range(10, -1, -1):
            csz = min(w, 1024)
            for k in range(w // csz):
                sl = slice(k * csz, (k + 1) * csz)
                su = slice(w + k * csz, w + (k + 1) * csz)
                ore, oim = re[:, sl], im[:, sl]
                tt0, tt1 = t0[:, 0:csz], t1[:, 0:csz]
                # upper half <- (c) * old  (written before old is clobbered)
                eng.tensor_scalar(tt0, ore, co(q, CO_CRE), None, ALU.mult)
                eng.scalar_tensor_tensor(re[:, su], oim, co(q, CO_MCIM),
                                         tt0, ALU.mult, ALU.add)
                eng.tensor_scalar(tt1, ore, co(q, CO_CIM), None, ALU.mult)
                eng.scalar_tensor_tensor(im[:, su], oim, co(q, CO_CRE),
                                         tt1, ALU.mult, ALU.add)
                # lower half <- (a) * old, in place
                eng.tensor_scalar(tt0, ore, co(q, CO_ARE), None, ALU.mult)
                eng.tensor_scalar(tt1, ore, co(q, CO_AIM), None, ALU.mult)
                eng.scalar_tensor_tensor(ore, oim, co(q, CO_MAIM),
                                         tt0, ALU.mult, ALU.add)
                eng.scalar_tensor_tensor(oim, oim, co(q, CO_ARE),
                                         tt1, ALU.mult, ALU.add)
            w *= 2

    def _observables(self, bt):
        """probs = re^2+im^2 (overwrites re plane), then the 12 <Z_q>."""
        nc = self.nc
        eng = nc.vector
        AF = mybir.ActivationFunctionType
        re = self.plane(bt, 0)
        im = self.plane(bt, 1)
        t0 = self.TS[0][0]
        t1 = self.TS[0][1]
        for h in range(4):
            sl = slice(h * 1024, (h + 1) * 1024)
            if SQUARES_ON_ACT:
                nc.scalar.activation(t0[:], re[:, sl], AF.Square)
                nc.scalar.activation(t1[:], im[:, sl], AF.Square)
            else:
                eng.tensor_tensor(t0[:], re[:, sl], re[:, sl], ALU.mult)
                eng.tensor_tensor(t1[:], im[:, sl], im[:, sl], ALU.mult)
            eng.tensor_tensor(re[:, sl], t0[:], t1[:], ALU.add)
        # fold out qubits MSB-first; z_q = sum(lo half) - sum(hi half)
        w = DIM
        for q in range(N_QUBITS):
            h = w // 2
            lo, hi = re[:, 0:h], re[:, h:w]
            if h > 1024:  # only q=0: do the diff/reduce in two chunks
                for k in range(2):
                    sk = slice(k * 1024, (k + 1) * 1024)
                    eng.tensor_tensor(t0[:], lo[:, sk], hi[:, sk],
                                      ALU.subtract)
                    eng.tensor_reduce(out=self.ZT[bt][:, 12 + k:13 + k],
                                      in_=t0[:], op=ALU.add,
                                      axis=mybir.AxisListType.X)
                eng.tensor_tensor(self.ZT[bt][:, q:q + 1],
                                  self.ZT[bt][:, 12:13],
                                  self.ZT[bt][:, 13:14], ALU.add)
            else:
                eng.tensor_tensor(t0[:, 0:h], lo, hi, ALU.subtract)
                eng.tensor_reduce(out=self.ZT[bt][:, q:q + 1],
                                  in_=t0[:, 0:h], op=ALU.add,
                                  axis=mybir.AxisListType.X)
            if q < N_QUBITS - 1:
                for k in range(max(1, h // 1024)):
                    sk = slice(k * 1024, min((k + 1) * 1024, h))
                    eng.tensor_tensor(lo[:, sk], lo[:, sk], hi[:, sk],
                                      ALU.add)
            w = h

    def _emit_circuit(self):
        for bt in range(NBT):
            self._kron_init(bt)
        for l in range(N_LAYERS):
            if l > 0:
                for q in range(N_QUBITS):
                    for bt in range(NBT):
                        self._gate_1q(bt, l, q)
            for c in range(N_QUBITS):
                self._crot_site(l, c)
        for bt in range(NBT):
            self._observables(bt)


_PROG_CACHE = None


def _get_prog():
    global _PROG_CACHE
    if _PROG_CACHE is None:
        _PROG_CACHE = _Prog()
    return _PROG_CACHE


def _run(inputs, trace=False):
    x = np.asarray(inputs["x"], np.float32)
    gco, cco = _host_coeffs(x, inputs["q_params_rot"], inputs["q_params_enta"])
    # gco: [L,Q,12,B] -> per-core [128, L*Q*12*NBT]
    in_maps = []
    cco_tile = np.broadcast_to(
        cco.reshape(1, CCO_W), (128, CCO_W)).copy()
    for core in range(N_CORES):
        lo = core * B_CORE
        g = gco[:, :, :, lo:lo + B_CORE]                 # [L,Q,12,512]
        g = g.reshape(N_LAYERS, N_QUBITS, NCO, NBT, 128)  # [L,Q,12,bt,p]
        g = np.ascontiguousarray(np.moveaxis(g, -1, 0))   # [p,L,Q,12,bt]
        in_maps.append({
            "gcoef": g.reshape(128, GCO_W),
            "ccoef": cco_tile,
            "ident": np.eye(128, dtype=np.float32),
        })
    prog = _get_prog()
    res = run_bass_kernel_spmd(prog.nc, in_maps, list(range(N_CORES)),
                               trace=trace)
    z = np.concatenate([res.results[c]["z"] for c in range(N_CORES)], axis=0)
    return z.astype(np.float32), res


def kernel(**inputs):
    z, _ = _run(inputs, trace=False)
    return z
